# revision 22
# baseline (speedup 1.0000x reference)
"""CrystalHypergraphConv on 8 Trainium2 NeuronCores (Bass/Tile).

Strategy:
- Nodes sharded by graph (batch is sorted); hyperedges sharded by range.
- Per layer: phase A computes hedge means (hm) of node features via
  indirect-DMA row gathers + PE one-hot segment-sum matmuls into PSUM
  windows of 128 segments; AllGather shares hm; phase B computes node
  means of hm rows the same way.  hedge_attr means (za) are layer-
  invariant and computed once with the same machinery.
- Dense tail (z @ lin_f/lin_c, gating, pooling, head) runs on PE/ACT/DVE
  with PE-transposed activation tiles.
All arithmetic in float32.
"""
import sys

for _p in ('/opt/trn_rl_repo',):
    if _p not in sys.path:
        sys.path.insert(0, _p)

import numpy as np

N, H, E, NG, HD, HF, HOUT, NL = 100000, 150000, 1800000, 256, 64, 35, 128, 3
NCORES = 8
P = 128

_CACHE = {}


def _ceil(a, b):
    return -(-a // b) * b


def _blocks_for_phase(seg_all, gidx_all, order, lo, hi, nwin, nbw):
    """Build fixed-shape [P, nwin*nbw] gather-index + seg-offset arrays.

    seg_all: per-incidence segment id (global); order: incidence order
    (sorted by segment); segments lo..hi-1 belong to this core and map to
    windows of 128.  Each window gets exactly nbw blocks of 128 slots,
    padded with (idx=0, seg=-1) dummies.
    """
    nb = nwin * nbw
    gidx = np.zeros((nb * P,), np.int32)
    segoff = np.full((nb * P,), -1.0, np.float32)
    segs = seg_all[order]
    win = (segs - lo) // P
    start = np.searchsorted(win, np.arange(nwin))
    end = np.searchsorted(win, np.arange(nwin), side='right')
    for w in range(nwin):
        s, e = start[w], end[w]
        cnt = e - s
        assert cnt <= nbw * P, f"window overflow {cnt} > {nbw * P}"
        base = w * nbw * P
        gidx[base:base + cnt] = gidx_all[order[s:e]]
        segoff[base:base + cnt] = (segs[s:e] - lo - w * P).astype(np.float32)
    # wrap to [P, nb]: slot i -> [i % P, i // P]
    return (gidx.reshape(nb, P).T.copy(), segoff.reshape(nb, P).T.copy())


def _host_prep(x, hedge_index, hedge_attr, batch):
    node_idx = np.asarray(hedge_index[0], np.int64)
    hedge_idx = np.asarray(hedge_index[1], np.int64)
    batch = np.asarray(batch, np.int64)

    gpc = NG // NCORES  # graphs per core
    node_start = np.searchsorted(batch, np.arange(0, NG + 1, gpc))  # [9]
    n_k = np.diff(node_start)
    n_pad = int(_ceil(int(n_k.max()), P))
    nwn = n_pad // P

    hpc = H // NCORES  # hedges per core (150000/8 = 18750)
    h_pad = int(_ceil(hpc, P))
    nwh = h_pad // P

    node_core = np.searchsorted(node_start[1:], np.arange(N), side='right')
    node_row = np.arange(N) - node_start[node_core] + node_core * n_pad
    hedge_row = (np.arange(H) % hpc) + (np.arange(H) // hpc) * h_pad

    deg_h = np.bincount(hedge_idx, minlength=H).astype(np.float32)
    deg_n = np.bincount(node_idx, minlength=N).astype(np.float32)

    cores = []
    nbw_a = nbw_b = 1
    pre = []
    for k in range(NCORES):
        # phase A: incidences whose hedge is in this core's shard
        mask = (hedge_idx >= k * hpc) & (hedge_idx < (k + 1) * hpc)
        ia = np.nonzero(mask)[0]
        ia = ia[np.argsort(hedge_idx[ia], kind='stable')]
        wa = (hedge_idx[ia] - k * hpc) // P
        ca = np.bincount(wa, minlength=nwh)
        nbw_a = max(nbw_a, int(_ceil(int(ca.max()), P)) // P)
        # phase B: incidences whose node is in this core's shard
        lo, hi = node_start[k], node_start[k + 1]
        mask = (node_idx >= lo) & (node_idx < hi)
        ib = np.nonzero(mask)[0]
        ib = ib[np.argsort(node_idx[ib], kind='stable')]
        wb = (node_idx[ib] - lo) // P
        cb = np.bincount(wb, minlength=nwn)
        nbw_b = max(nbw_b, int(_ceil(int(cb.max()), P)) // P)
        pre.append((ia, ib))

    for k in range(NCORES):
        ia, ib = pre[k]
        lo, hi = int(node_start[k]), int(node_start[k + 1])
        nk = hi - lo
        gA, sA = _blocks_for_phase(hedge_idx, node_row[node_idx], ia,
                                   k * hpc, (k + 1) * hpc, nwh, nbw_a)
        gBh, sB = _blocks_for_phase(node_idx, hedge_row[hedge_idx], ib,
                                    lo, hi, nwn, nbw_b)
        gBa, _ = _blocks_for_phase(node_idx, hedge_idx.astype(np.int64), ib,
                                   lo, hi, nwn, nbw_b)
        # inverse counts laid out [P, nwin]
        invA = np.zeros((P, nwh), np.float32)
        dh = deg_h[k * hpc:(k + 1) * hpc]
        invA.T.flat[:len(dh)] = 1.0 / np.maximum(dh, 1.0)
        invB = np.zeros((P, nwn), np.float32)
        dn = deg_n[lo:hi]
        invB.T.flat[:nk] = 1.0 / np.maximum(dn, 1.0)
        # pooling one-hot [P, nwn*gpc] and inverse graph sizes
        g_of = np.full((n_pad,), -1, np.int64)
        g_of[:nk] = batch[lo:hi] - k * gpc
        po = np.zeros((P, nwn * gpc), np.float32)
        for t in range(nwn):
            blk = g_of[t * P:(t + 1) * P]
            oh = np.zeros((P, gpc), np.float32)
            valid = blk >= 0
            oh[np.nonzero(valid)[0], blk[valid]] = 1.0
            po[:, t * gpc:(t + 1) * gpc] = oh
        cnt_g = np.bincount(batch, minlength=NG)[k * gpc:(k + 1) * gpc]
        invP = np.zeros((P, 1), np.float32)
        invP[:gpc, 0] = 1.0 / np.maximum(cnt_g, 1)
        xs = np.zeros((n_pad, 92), np.float32)
        xs[:nk] = x[lo:hi]
        cores.append(dict(x_shard=xs, gidxA=gA, segA=sA, gidxBh=gBh,
                          gidxBa=gBa, segB=sB, invA=invA, invB=invB,
                          poolOH=po, invP=invP))
    meta = dict(n_pad=n_pad, nwn=nwn, h_pad=h_pad, nwh=nwh,
                nbw_a=nbw_a, nbw_b=nbw_b, gpc=gpc)
    return cores, meta


def _build_nc(meta):
    from concourse import bass, mybir, tile, bacc
    from concourse.masks import make_identity
    f32 = mybir.dt.float32
    AF = mybir.ActivationFunctionType
    OP = mybir.AluOpType

    n_pad, nwn = meta['n_pad'], meta['nwn']
    h_pad, nwh = meta['h_pad'], meta['nwh']
    nbw_a, nbw_b, gpc = meta['nbw_a'], meta['nbw_b'], meta['gpc']
    nbA, nbB = nwh * nbw_a, nwn * nbw_b

    nc = bacc.Bacc("TRN2", target_bir_lowering=False, debug=False,
                   num_devices=NCORES)

    def inp(name, shape):
        return nc.dram_tensor(name, shape, f32, kind="ExternalInput").ap()

    x_t = inp("x_shard", [n_pad, 92])
    attr_t = inp("hedge_attr", [H, HF])
    gidxA_t = nc.dram_tensor("gidxA", [P, nbA], mybir.dt.int32, kind="ExternalInput").ap()
    gidxBh_t = nc.dram_tensor("gidxBh", [P, nbB], mybir.dt.int32, kind="ExternalInput").ap()
    gidxBa_t = nc.dram_tensor("gidxBa", [P, nbB], mybir.dt.int32, kind="ExternalInput").ap()
    segA_t = inp("segA", [P, nbA])
    segB_t = inp("segB", [P, nbB])
    invA_t = inp("invA", [P, nwh])
    invB_t = inp("invB", [P, nwn])
    poolOH_t = inp("poolOH", [P, nwn * gpc])
    invP_t = inp("invP", [P, 1])
    iota_t = inp("iota", [P, P])
    embw_t = inp("embed_w", [92, HD])
    embb_t = inp("embed_b", [P, HD])
    linf_t = inp("lin_f_w", [NL, 163, HD])
    linfb_t = inp("lin_f_b", [NL, P, HD])
    linc_t = inp("lin_c_w", [NL, 163, HD])
    lincb_t = inp("lin_c_b", [NL, P, HD])
    projw_t = inp("proj_w", [HD, HOUT])
    projb_t = inp("proj_b", [P, HOUT])
    outw_t = inp("out_w", [HOUT, 1])
    y_t = nc.dram_tensor("y", [gpc, 1], f32, kind="ExternalOutput").ap()

    h_table = nc.dram_tensor("h_table", [NCORES * n_pad, HD], f32).ap()
    hm_table = nc.dram_tensor("hm_table", [NCORES * h_pad, HD], f32).ap()
    hsh = nc.dram_tensor("hsh", [n_pad, HD], f32).ap()
    hmsh = nc.dram_tensor("hmsh", [h_pad, HD], f32).ap()

    with tile.TileContext(nc) as tc:
        with (
            tc.tile_pool(name="const", bufs=1) as cpool,
            tc.tile_pool(name="sb", bufs=1) as sb,
            tc.tile_pool(name="gt", bufs=8) as gpool,
            tc.tile_pool(name="oh", bufs=4) as ohpool,
            tc.tile_pool(name="work", bufs=4) as wpool,
            tc.tile_pool(name="ps", bufs=2, space="PSUM") as pspool,
            tc.tile_pool(name="pst", bufs=2, space="PSUM") as pstpool,
            tc.tile_pool(name="psw", bufs=3, space="PSUM") as pswpool,
        ):
            ident = cpool.tile([P, P], f32, name="ident")
            make_identity(nc, ident[:])
            iota = cpool.tile([P, P], f32, name="iota")
            nc.sync.dma_start(out=iota[:], in_=iota_t[:])

            def load_const(name, t, shape):
                tl = cpool.tile(shape, t.dtype if hasattr(t, 'dtype') else f32, name=name)
                nc.sync.dma_start(out=tl[:], in_=t[:])
                return tl

            gidxA = load_const("gidxA", gidxA_t, [P, nbA])
            segA = load_const("segA", segA_t, [P, nbA])
            gidxBh = load_const("gidxBh", gidxBh_t, [P, nbB])
            gidxBa = load_const("gidxBa", gidxBa_t, [P, nbB])
            segB = load_const("segB", segB_t, [P, nbB])
            invA = load_const("invA", invA_t, [P, nwh])
            invB = load_const("invB", invB_t, [P, nwn])
            poolOH = load_const("poolOH", poolOH_t, [P, nwn * gpc])
            invP = load_const("invP", invP_t, [P, 1])
            embw = load_const("embw", embw_t, [92, HD])
            embb = load_const("embb", embb_t, [P, HD])
            projw = load_const("projw", projw_t, [HD, HOUT])
            projb = load_const("projb", projb_t, [P, HOUT])
            outw = load_const("outw", outw_t, [HOUT, 1])
            def load_lin(name, t):
                # K-chunks: rows 0:HD (h), HD:HD+HF (za), HD+HF:163 (zb)
                return [load_const(f"{name}a", t[0:HD], [HD, HD]),
                        load_const(f"{name}b", t[HD:HD + HF], [HF, HD]),
                        load_const(f"{name}c", t[HD + HF:163], [HD, HD])]

            linf = [load_lin(f"linf{l}", linf_t[l]) for l in range(NL)]
            linfb = [load_const(f"linfb{l}", linfb_t[l], [P, HD]) for l in range(NL)]
            linc = [load_lin(f"linc{l}", linc_t[l]) for l in range(NL)]
            lincb = [load_const(f"lincb{l}", lincb_t[l], [P, HD]) for l in range(NL)]

            h_sb = sb.tile([P, nwn * HD], f32, name="h_sb")
            za_sb = sb.tile([P, nwn * HF], f32, name="za_sb")
            zb_sb = sb.tile([P, nwn * HD], f32, name="zb_sb")

            def softplus(dst_ap, src_ap, pp_, ff):
                """dst = relu(src) + ln(1 + exp(-|src|)) (stable softplus)."""
                t1 = wpool.tile([P, ff], f32, name="spt1", tag="spt1")
                t2 = wpool.tile([P, ff], f32, name="spt2", tag="spt2")
                nc.scalar.activation(t1[:pp_, :], src_ap, AF.Abs)
                nc.scalar.activation(t1[:pp_, :], t1[:pp_, :], AF.Exp, scale=-1.0)
                nc.scalar.activation(t1[:pp_, :], t1[:pp_, :], AF.Ln, bias=1.0)
                nc.scalar.activation(t2[:pp_, :], src_ap, AF.Relu)
                nc.vector.tensor_tensor(out=dst_ap, in0=t1[:pp_, :], in1=t2[:pp_, :],
                                        op=OP.add)

            def transpose_to(dst_sb, src_ap, rows, cols):
                """dst_sb[:cols, :rows] = src_ap.T via PE."""
                pt = pstpool.tile([P, P], f32, name="pt")
                nc.tensor.transpose(out=pt[:cols, :rows], in_=src_ap,
                                    identity=ident[:rows, :rows])
                nc.vector.tensor_copy(out=dst_sb[:cols, :rows], in_=pt[:cols, :rows])

            # ---------- embed ----------
            for t in range(nwn):
                xt = wpool.tile([P, 92], f32, name="xt", tag="xt")
                nc.sync.dma_start(out=xt[:], in_=x_t[t * P:(t + 1) * P, :])
                xT = wpool.tile([P, P], f32, name="xT", tag="xT")
                transpose_to(xT, xt[:], P, 92)
                ph = pspool.tile([P, HD], f32, name="ph", tag="dense")
                nc.tensor.matmul(out=ph[:], lhsT=xT[:92, :], rhs=embw[:92, :],
                                 start=True, stop=True)
                nc.vector.tensor_tensor(out=h_sb[:, t * HD:(t + 1) * HD], in0=ph[:],
                                        in1=embb[:],
                                        op=OP.add)
            def share_h():
                nc.sync.dma_start(out=hsh[:].rearrange("(t p) d -> p t d", p=P),
                                  in_=h_sb[:].rearrange("p (t d) -> p t d", d=HD))
                nc.gpsimd.collective_compute(
                    "AllGather", OP.bypass, replica_groups=[list(range(NCORES))],
                    ins=[hsh[:]], outs=[h_table[:]])

            share_h()

            def seg_phase(nwin, nbw, gidx_sb, seg_sb, table_ap, width, inv_sb, emit):
                """Windowed gather + one-hot segment-sum.

                emit(w, sums_ap): sums_ap is SBUF [P, width] of per-segment
                means for window w."""
                for w in range(nwin):
                    pw = pswpool.tile([P, width], f32, name="pw", tag="segpsum")
                    for j in range(nbw):
                        b = w * nbw + j
                        gt = gpool.tile([P, width], f32, name="gt", tag="gt")
                        nc.gpsimd.indirect_dma_start(
                            out=gt[:], out_offset=None, in_=table_ap,
                            in_offset=bass.IndirectOffsetOnAxis(
                                ap=gidx_sb[:, b:b + 1], axis=0))
                        oh = ohpool.tile([P, P], f32, name="oh", tag="oh")
                        nc.vector.tensor_tensor(
                            out=oh[:], in0=seg_sb[:, b:b + 1].to_broadcast([P, P]),
                            in1=iota[:], op=OP.is_equal)
                        nc.tensor.matmul(out=pw[:], lhsT=oh[:], rhs=gt[:],
                                         start=(j == 0), stop=(j == nbw - 1))
                    sm = wpool.tile([P, width], f32, name="sm", tag="segout")
                    nc.vector.tensor_scalar_mul(sm[:], pw[:], inv_sb[:, w:w + 1])
                    emit(w, sm)

            # ---------- za (layer-invariant) ----------
            def emit_za(w, sm):
                nc.vector.tensor_copy(out=za_sb[:, w * HF:(w + 1) * HF], in_=sm[:])

            seg_phase(nwn, nbw_b, gidxBa, segB, attr_t[:], HF, invB, emit_za)

            # ---------- layers ----------
            for l in range(NL):
                # phase A: hedge means of h
                def emit_hm(w, sm):
                    nc.sync.dma_start(out=hmsh[w * P:(w + 1) * P, :], in_=sm[:])

                seg_phase(nwh, nbw_a, gidxA, segA, h_table[:], HD, invA, emit_hm)
                nc.gpsimd.collective_compute(
                    "AllGather", OP.bypass, replica_groups=[list(range(NCORES))],
                    ins=[hmsh[:]], outs=[hm_table[:]])

                # phase B: node means of hm
                def emit_zb(w, sm):
                    nc.vector.tensor_copy(out=zb_sb[:, w * HD:(w + 1) * HD], in_=sm[:])

                seg_phase(nwn, nbw_b, gidxBh, segB, hm_table[:], HD, invB, emit_zb)

                # tail: z = [h | za | zb];  h = softplus(sigmoid(zf)*softplus(zc)+h)
                for t in range(nwn):
                    hT = wpool.tile([P, P], f32, name="hT", tag="hT")
                    transpose_to(hT, h_sb[:, t * HD:(t + 1) * HD], P, HD)
                    aT = wpool.tile([P, P], f32, name="aT", tag="aT")
                    transpose_to(aT, za_sb[:, t * HF:(t + 1) * HF], P, HF)
                    bT = wpool.tile([P, P], f32, name="bT", tag="bT")
                    transpose_to(bT, zb_sb[:, t * HD:(t + 1) * HD], P, HD)
                    pf = pspool.tile([P, HD], f32, name="pf", tag="dense")
                    pc = pspool.tile([P, HD], f32, name="pc", tag="dense")
                    for (ps, lw) in ((pf, linf[l]), (pc, linc[l])):
                        nc.tensor.matmul(out=ps[:], lhsT=hT[:HD, :], rhs=lw[0][:],
                                         start=True, stop=False)
                        nc.tensor.matmul(out=ps[:], lhsT=aT[:HF, :],
                                         rhs=lw[1][:], start=False, stop=False)
                        nc.tensor.matmul(out=ps[:], lhsT=bT[:HD, :],
                                         rhs=lw[2][:], start=False, stop=True)
                    zf = wpool.tile([P, HD], f32, name="zf", tag="zf")
                    nc.vector.tensor_tensor(out=zf[:], in0=pf[:],
                                            in1=linfb[l][:], op=OP.add)
                    sig = wpool.tile([P, HD], f32, name="sig", tag="sig")
                    nc.scalar.activation(sig[:], zf[:], AF.Sigmoid)
                    zc = wpool.tile([P, HD], f32, name="zc", tag="zc")
                    nc.vector.tensor_tensor(out=zc[:], in0=pc[:],
                                            in1=lincb[l][:], op=OP.add)
                    spc = wpool.tile([P, HD], f32, name="spc", tag="spc")
                    softplus(spc[:], zc[:], P, HD)
                    prod = wpool.tile([P, HD], f32, name="prod", tag="prod")
                    nc.vector.tensor_tensor(out=prod[:], in0=sig[:], in1=spc[:], op=OP.mult)
                    nc.vector.tensor_tensor(out=prod[:], in0=prod[:],
                                            in1=h_sb[:, t * HD:(t + 1) * HD], op=OP.add)
                    softplus(h_sb[:, t * HD:(t + 1) * HD], prod[:], P, HD)
                if l < NL - 1:
                    share_h()

            # ---------- pooling + head ----------
            pp = pspool.tile([P, HD], f32, name="pp", tag="dense")
            for t in range(nwn):
                nc.tensor.matmul(out=pp[:gpc, :], lhsT=poolOH[:, t * gpc:(t + 1) * gpc],
                                 rhs=h_sb[:, t * HD:(t + 1) * HD],
                                 start=(t == 0), stop=(t == nwn - 1))
            pooled = wpool.tile([P, HD], f32, name="pooled")
            nc.vector.tensor_scalar_mul(pooled[:gpc, :], pp[:gpc, :], invP[:gpc, :])
            pT = wpool.tile([P, P], f32, name="pT")
            transpose_to(pT, pooled[:gpc, :], gpc, HD)
            pg = pspool.tile([P, HOUT], f32, name="pg", tag="dense")
            nc.tensor.matmul(out=pg[:gpc, :], lhsT=pT[:HD, :gpc], rhs=projw[:],
                             start=True, stop=True)
            gsb = wpool.tile([P, HOUT], f32, name="gsb")
            nc.vector.tensor_tensor(out=gsb[:gpc, :], in0=pg[:gpc, :],
                                    in1=projb[:gpc, :], op=OP.add)
            softplus(gsb[:gpc, :], gsb[:gpc, :], gpc, HOUT)
            gT = wpool.tile([P, P], f32, name="gT")
            transpose_to(gT, gsb[:gpc, :], gpc, HOUT)
            py = pspool.tile([P, 1], f32, name="py", tag="dense")
            nc.tensor.matmul(out=py[:gpc, :], lhsT=gT[:HOUT, :gpc], rhs=outw[:],
                             start=True, stop=True)
            ysb = wpool.tile([P, 1], f32, name="ysb")
            nc.vector.tensor_copy(out=ysb[:gpc, :], in_=py[:gpc, :])
            nc.sync.dma_start(out=y_t[:], in_=ysb[:gpc, :])

    nc.compile()
    return nc


def kernel(x, hedge_index, hedge_attr, batch,
           embed_w, embed_b, lin_f_w, lin_f_b, lin_c_w, lin_c_b,
           proj_w, proj_b, out_w, out_b):
    from concourse.bass_utils import run_bass_kernel_spmd

    x = np.asarray(x, np.float32)
    hedge_attr = np.asarray(hedge_attr, np.float32)
    cores, meta = _host_prep(x, np.asarray(hedge_index), hedge_attr,
                             np.asarray(batch))

    key = (meta['n_pad'], meta['nwh'], meta['nbw_a'], meta['nbw_b'])
    if key not in _CACHE:
        _CACHE[key] = _build_nc(meta)
    nc = _CACHE[key]

    shared = dict(
        hedge_attr=hedge_attr,
        iota=np.tile(np.arange(P, dtype=np.float32), (P, 1)),
        embed_w=np.asarray(embed_w, np.float32),
        embed_b=np.tile(np.asarray(embed_b, np.float32).reshape(1, HD), (P, 1)),
        lin_f_w=np.asarray(lin_f_w, np.float32),
        lin_f_b=np.tile(np.asarray(lin_f_b, np.float32).reshape(NL, 1, HD), (1, P, 1)),
        lin_c_w=np.asarray(lin_c_w, np.float32),
        lin_c_b=np.tile(np.asarray(lin_c_b, np.float32).reshape(NL, 1, HD), (1, P, 1)),
        proj_w=np.asarray(proj_w, np.float32),
        proj_b=np.tile(np.asarray(proj_b, np.float32).reshape(1, HOUT), (P, 1)),
        out_w=np.asarray(out_w, np.float32),
    )
    in_maps = [dict(shared, **cores[k]) for k in range(NCORES)]
    res = None
    for attempt in range(3):
        try:
            res = run_bass_kernel_spmd(nc, in_maps, core_ids=list(range(NCORES)))
            break
        except Exception:
            if attempt == 2:
                raise
    y = np.concatenate([res.results[k]["y"] for k in range(NCORES)], axis=0)
    return (y + np.asarray(out_b, np.float32).reshape(1, 1)).astype(np.float32)
